# revision 6
# baseline (speedup 1.0000x reference)
"""Conv2d 3x3 (pad 1, stride 1) + bias on 8 Trainium2 cores.

Problem: x [32,128,56,56] f32, weights [256,128,3,3] f32, bias [256] f32
         -> out [32,256,56,56] f32.

Strategy
--------
Data-parallel over batch: each of the 8 cores owns 4 images.

Per core, implicit GEMM on a shared-padding row layout with stride 57:
  - Padded pixel (ih, iw), ih in [-1,56], iw in [-1,56], lives at flat
    index (ih+1)*57 + (iw+1); the right pad of row r IS the left pad of
    row r+1 (one shared zero column), so the buffer is 58*57+1(+1) = 3308
    bf16 per channel. The host builds this layout with np.pad.
  - Output is computed in the same stride-57 layout: out position
    p = oh*57 + ow. Every tap (kh, kw) of the 3x3 kernel is then a
    CONSTANT offset kh*57+kw into the flat padded input, so one matmul
    covers 8 output rows at once (N = 8*57 = 456 <= 512 PSUM bank).

bf16 + tap-outer (vs the f32r tap-inner baseline at 123.8us):
  - f32r matmuls are SELF-LOADING: the 128-cycle weight load serializes
    with every matmul (128/2.4GHz + 2.5ns NX = 55.8ns/mm, exactly the
    measured baseline gap).  bf16 matmuls emit a separate LDWEIGHTS that
    the PE's 64-deep reorder window pulls ahead into the background
    weight buffer while the previous matmul streams: measured steady
    state is 192-196 ns/matmul = the N/2.4GHz + NX roofline.
  - Tap-outer ordering: for each (image, cout-half) the 9 taps are the
    outer loop and the 7 output tiles the inner loop, so 7 consecutive
    matmuls share one stationary weight tile (7 PSUM banks live, bank t
    recycles exactly when the next group's tap-0 sweep reaches tile t).
  - bf16 end-to-end (hostside convert): max rel err ~3.5e-3 (vs 2e-2
    gate), and input/output DMA bytes halve.
  - PSUM tiles drain with the bias add fused: even tiles on DVE
    (tensor_scalar_add), odd tiles on ACT (activation Identity+bias), so
    the drain never gates the next group's first matmuls.

DMA schedule (16 shared HW engines, ~0.8-1.8us queue-start latency,
~300 GB/s aggregate):
  - The SYNC queue starts earliest after the ~7.2us framework preamble,
    so it carries everything the first matmuls gate on: w taps 0-2 of
    half 0 first, then image 0 in four chunks sized to the pair-group
    consumption order.  Remaining weights + bias go on the ACT queue;
    images 1-3 prefetch whole on the SYNC queue a full image ahead.
  - (b=0, h=0) is computed in pair-groups (tap-outer within each pair)
    so the first matmuls gate on ~1/3 of image 0.
  - Outputs ship as 2 DMAs per group (tiles 0-3, 4-6) alternating the
    GPSIMD and SYNC queues mid-run.  The LAST group ships 4 smaller
    blocks on 4 different queues (gpsimd/sync/vector/scalar) so the
    kernel tail is one small transfer, not 817KB serialized behind one
    queue: measured tail drops from ~8.9us to ~3us.
  - PE warmup: a few bf16 throwaway matmuls on a memset tile keep the PE
    busy from t=0 so the HAM clock gate flips to 8/8 (2.4 GHz) ~3.4us
    after the first warmup while real (cold) matmuls make progress.

Built on bacc.Bacc: walrus engine-instruction structs hold at most ONE
sync wait; Bacc's compile() runs move_matmul_waits_to_ldweights /
generate_event_semaphores to split excess waits.
"""

import numpy as np
import ml_dtypes

import concourse.bacc as bacc
import concourse.mybir as mybir
import concourse.tile as tile
from concourse.bass_utils import run_bass_kernel_spmd

B, CIN, H, W = 32, 128, 56, 56
COUT = 256
NCORES = 8
BLOC = B // NCORES  # images per core
SP = W + 1  # 57: row stride of the shared-padding layout
# One trailing zero for the 58*57+1 layout plus one more so the last
# tile's bottom-right tap (offset 2*57+2) stays in bounds: 6*456+116+456.
NPIX = (H + 2) * SP + 2  # 3308 padded bf16 per channel
OUTW = H * SP  # 3192 output cols in stride-57 layout (1 junk col per row)
TILE_N = 8 * SP  # 456: 8 output rows per PSUM tile
NTILES = 7  # 7 x 456 = 3192
WARMUP = 5  # throwaway matmuls covering DMA-start latency on the PE
# Image-0 chunks on the SYNC queue; pair (t0,t0+1) needs cols <
# 456*(t0+2)+116, chunk k arrives comfortably before pair k consumes it.
XBOUNDS = [0, 520, 1368, 2216, NPIX]

_nc_cache = None


def _build():
    f32 = mybir.dt.float32
    bf16 = mybir.dt.bfloat16
    nc = bacc.Bacc("TRN2", target_bir_lowering=False)
    x_d = nc.dram_tensor("xp", [BLOC, CIN, NPIX], bf16, kind="ExternalInput")
    w_d = nc.dram_tensor("wT", [CIN, 9 * COUT], bf16, kind="ExternalInput")
    b_d = nc.dram_tensor("bias2", [128, 2], f32, kind="ExternalInput")
    o_d = nc.dram_tensor("out", [BLOC, COUT, OUTW], bf16, kind="ExternalOutput")

    with tile.TileContext(nc) as tc:
        with (
            tc.tile_pool(name="wpool", bufs=1) as wpool,
            tc.tile_pool(name="xpool", bufs=2) as xpool,
            tc.tile_pool(name="opool", bufs=4) as opool,
            tc.tile_pool(name="psum", bufs=1, space="PSUM") as psum,
            tc.tile_pool(name="wupsum", bufs=1, space="PSUM") as wupsum,
        ):
            wsb = wpool.tile([CIN, 9 * COUT], bf16)
            bsb = wpool.tile([128, 2], f32)
            # First trigger on the early-starting SYNC queue: the weights
            # the very first matmuls need (half 0, taps 0-2).
            nc.sync.dma_start(wsb[:, :384], w_d[:, :384])
            # Rest of the weights + bias on the ACT queue (starts ~1us
            # later, still well before tap 3 of the first group).
            nc.scalar.dma_start(wsb[:, 384:1152], w_d[:, 384:1152])
            nc.scalar.dma_start(wsb[:, 1152:], w_d[:, 1152:])
            nc.scalar.dma_start(bsb[:], b_d[:])

            # PE warmup (see module docstring).
            wub = wpool.tile([128, 512], bf16)
            nc.vector.memset(wub[:], 0.0)
            wup = wupsum.tile([128, 512], f32)
            for _ in range(WARMUP):
                nc.tensor.matmul(
                    wup[:], lhsT=wub[:, :128], rhs=wub[:],
                    start=True, stop=True,
                )

            for b in range(BLOC):
                xp = xpool.tile([CIN, NPIX], bf16, tag="xp")
                bounds = XBOUNDS if b == 0 else [0, NPIX]
                for lo, hi in zip(bounds, bounds[1:]):
                    nc.sync.dma_start(xp[:, lo:hi], x_d[b, :, lo:hi])
                for h in range(2):
                    first = b == 0 and h == 0
                    last = b == BLOC - 1 and h == 1
                    # Compute sub-groups: pairs while image 0 streams in.
                    cgroups = (
                        [[0, 1], [2, 3], [4, 5], [6]]
                        if first
                        else [list(range(NTILES))]
                    )
                    # Output blocks (tiles, dma-queue engine).
                    if first:
                        oplan = [([0, 1], nc.gpsimd), ([2, 3], nc.sync),
                                 ([4, 5], nc.gpsimd), ([6], nc.sync)]
                    elif last:
                        oplan = [([0, 1], nc.gpsimd), ([2, 3], nc.gpsimd),
                                 ([4, 5], nc.sync), ([6], nc.scalar)]
                    else:
                        qa, qb = (nc.gpsimd, nc.sync) if h == 0 else (nc.sync, nc.gpsimd)
                        oplan = [([0, 1, 2, 3], qa), ([4, 5, 6], qb)]

                    pts = {}
                    for grp in cgroups:
                        for t in grp:
                            pts[t] = psum.tile(
                                [128, TILE_N], f32, tag=f"pt{t}", name=f"pt{t}"
                            )
                        # Tap-outer: one stationary weight tile feeds all
                        # tiles of the group before the next tap loads.
                        for tap in range(9):
                            kh, kw = divmod(tap, 3)
                            c0 = h * (9 * 128) + tap * 128
                            for t in grp:
                                off = t * TILE_N + kh * SP + kw
                                nc.tensor.matmul(
                                    pts[t][:],
                                    lhsT=wsb[:, c0 : c0 + 128],
                                    rhs=xp[:, off : off + TILE_N],
                                    start=(tap == 0),
                                    stop=(tap == 8),
                                )
                        # Drain this sub-group's finished blocks: bias-add
                        # PSUM -> SBUF bf16, even tiles on DVE, odd on ACT,
                        # then ship each completed output block.
                        for ts_, q in oplan:
                            if not all(t in grp for t in ts_):
                                continue
                            ot = opool.tile(
                                [128, 4 * TILE_N], bf16, tag="ot", name="ot"
                            )
                            for j, t in enumerate(ts_):
                                dst = ot[:, j * TILE_N : (j + 1) * TILE_N]
                                if t % 2 == 0:
                                    nc.vector.tensor_scalar_add(
                                        dst, pts[t][:], bsb[:, h : h + 1]
                                    )
                                else:
                                    nc.scalar.add(
                                        dst, pts[t][:], bsb[:, h : h + 1]
                                    )
                            w_out = len(ts_) * TILE_N
                            q.dma_start(
                                o_d[b, h * 128 : (h + 1) * 128,
                                    ts_[0] * TILE_N : ts_[0] * TILE_N + w_out],
                                ot[:, :w_out],
                            )
    nc.compile()
    return nc


def _get_nc():
    global _nc_cache
    if _nc_cache is None:
        _nc_cache = _build()
    return _nc_cache


def _prep_inputs(x, weights, bias):
    x = np.ascontiguousarray(np.asarray(x, dtype=np.float32))
    weights = np.ascontiguousarray(np.asarray(weights, dtype=np.float32))
    bias = np.ascontiguousarray(np.asarray(bias, dtype=np.float32))

    # Shared-padding stride-57 layout: rows -1..56 at stride 57 plus one
    # trailing zero (the last row's right pad) plus one slack zero.
    xpad = np.pad(x, ((0, 0), (0, 0), (1, 1), (1, 0))).reshape(B, CIN, (H + 2) * SP)
    xp = np.concatenate(
        [xpad, np.zeros((B, CIN, 2), dtype=np.float32)], axis=2
    ).astype(ml_dtypes.bfloat16)  # [B, CIN, 3308]
    # [Cout,Cin,3,3] -> [Cin, (half kh kw co)] so each Cout-half's taps are
    # one contiguous 1152-col block and each lhsT tap slice is contiguous.
    wT = np.ascontiguousarray(
        weights.reshape(2, 128, CIN, 3, 3).transpose(2, 0, 3, 4, 1)
    ).reshape(CIN, 9 * COUT).astype(ml_dtypes.bfloat16)
    b2 = np.ascontiguousarray(bias.reshape(2, 128).T)  # b2[p, h] = bias[h*128+p]

    return [
        {
            "xp": np.ascontiguousarray(xp[i * BLOC : (i + 1) * BLOC]),
            "wT": wT,
            "bias2": b2,
        }
        for i in range(NCORES)
    ]


def _run(inputs, trace=False):
    in_maps = _prep_inputs(inputs["x"], inputs["weights"], inputs["bias"])
    res = run_bass_kernel_spmd(
        _get_nc(), in_maps, core_ids=list(range(NCORES)), trace=trace
    )
    out = np.concatenate(
        [r["out"].astype(np.float32) for r in res.results], axis=0
    )  # [B, COUT, 3192]
    out = out.reshape(B, COUT, H, SP)[:, :, :, :W]
    return np.ascontiguousarray(out), res


def kernel(x, weights, bias):
    out, _ = _run({"x": x, "weights": weights, "bias": bias})
    return out


# revision 7
# speedup vs baseline: 1.0399x; 1.0399x over previous
"""Conv2d 3x3 (pad 1, stride 1) + bias on 8 Trainium2 cores.

Problem: x [32,128,56,56] f32, weights [256,128,3,3] f32, bias [256] f32
         -> out [32,256,56,56] f32.

Strategy
--------
Data-parallel over batch: each of the 8 cores owns 4 images.

Per core, implicit GEMM on a shared-padding row layout with stride 57:
  - Padded pixel (ih, iw), ih in [-1,56], iw in [-1,56], lives at flat
    index (ih+1)*57 + (iw+1); the right pad of row r IS the left pad of
    row r+1 (one shared zero column), so the buffer is 58*57+1(+1) = 3308
    bf16 per channel. The host builds this layout with np.pad.
  - Output is computed in the same stride-57 layout: out position
    p = oh*57 + ow. Every tap (kh, kw) of the 3x3 kernel is then a
    CONSTANT offset kh*57+kw into the flat padded input, so one matmul
    covers 8 output rows at once (N = 8*57 = 456 <= 512 PSUM bank).

bf16 + tap-outer (vs the f32r tap-inner baseline at 123.8us):
  - f32r matmuls are SELF-LOADING: the 128-cycle weight load serializes
    with every matmul (128/2.4GHz + 2.5ns NX = 55.8ns/mm, exactly the
    measured baseline gap).  bf16 matmuls emit a separate LDWEIGHTS that
    the PE's 64-deep reorder window pulls ahead into the background
    weight buffer while the previous matmul streams: measured steady
    state is 192-196 ns/matmul = the N/2.4GHz + NX roofline.
  - Tap-outer ordering: for each (image, cout-half) the 9 taps are the
    outer loop and the 7 output tiles the inner loop, so 7 consecutive
    matmuls share one stationary weight tile (7 PSUM banks live, bank t
    recycles exactly when the next group's tap-0 sweep reaches tile t).
  - bf16 end-to-end (hostside convert): max rel err ~3.5e-3 (vs 2e-2
    gate), and input/output DMA bytes halve.
  - PSUM tiles drain with the bias add fused: even tiles on DVE
    (tensor_scalar_add), odd tiles on ACT (activation Identity+bias), so
    the drain never gates the next group's first matmuls.

DMA schedule (16 shared SDMA engines, ~1us queue-start latency, ~300
GB/s aggregate; HWDGE rings on sync/scalar ~0.6us fixed, SWDGE on
gpsimd ~2us fixed):
  - Tile's dep tracking gates a tile's readers on ALL outstanding writes
    to it, so chunking one image DMA does NOT let early matmuls start
    early (measured: first matmul waited for the last chunk).  Image 0
    therefore lands as FOUR SEPARATE overlapping tiles (cols p*912 ..
    p*912+1144) and tile t reads piece t//2 only: the first matmuls
    gate on piece 0 alone.
  - The SYNC queue starts earliest after the ~7.2us framework preamble,
    so it carries w taps 0-2 of half 0 first, then image-0 pieces, then
    whole-image prefetches for images 1-3.  Remaining weights + bias go
    on the ACT queue.
  - (b=0, h=0) and the LAST (b,h) are computed in pair-groups; outputs
    ship per pair there, and as 2 blocks (tiles 0-3 / 4-6) otherwise,
    spread over the gpsimd and sync queues.  The last group's blocks go
    on HWDGE queues only (sync/sync/gpsimd-early/scalar-last) so the
    kernel tail is one small low-latency transfer.
  - PE warmup: a few bf16 throwaway matmuls on a memset tile keep the PE
    busy from t=0 so the HAM clock gate flips to 8/8 (2.4 GHz) ~3.4us
    after the first warmup while real (cold) matmuls make progress.

Built on bacc.Bacc: walrus engine-instruction structs hold at most ONE
sync wait; Bacc's compile() runs move_matmul_waits_to_ldweights /
generate_event_semaphores to split excess waits.
"""

import numpy as np
import ml_dtypes

import concourse.bacc as bacc
import concourse.mybir as mybir
import concourse.tile as tile
from concourse.bass_utils import run_bass_kernel_spmd

B, CIN, H, W = 32, 128, 56, 56
COUT = 256
NCORES = 8
BLOC = B // NCORES  # images per core
SP = W + 1  # 57: row stride of the shared-padding layout
# One trailing zero for the 58*57+1 layout plus one more so the last
# tile's bottom-right tap (offset 2*57+2) stays in bounds: 6*456+116+456.
NPIX = (H + 2) * SP + 2  # 3308 padded bf16 per channel
OUTW = H * SP  # 3192 output cols in stride-57 layout (1 junk col per row)
TILE_N = 8 * SP  # 456: 8 output rows per PSUM tile
NTILES = 7  # 7 x 456 = 3192
WARMUP = 5  # throwaway matmuls covering DMA-start latency on the PE
# Image-0 pieces: piece p holds cols [p*912, p*912+1144) (last: 3308);
# tile t (reads cols [456t, 456t+572)) lives entirely in piece t//2.
PIECE = 2 * TILE_N  # 912
PBOUNDS = [(0, 1144), (912, 2056), (1824, 2968), (2736, NPIX)]

_nc_cache = None


def _build():
    f32 = mybir.dt.float32
    bf16 = mybir.dt.bfloat16
    nc = bacc.Bacc("TRN2", target_bir_lowering=False)
    x_d = nc.dram_tensor("xp", [BLOC, CIN, NPIX], bf16, kind="ExternalInput")
    w_d = nc.dram_tensor("wT", [CIN, 9 * COUT], bf16, kind="ExternalInput")
    b_d = nc.dram_tensor("bias2", [128, 2], f32, kind="ExternalInput")
    o_d = nc.dram_tensor("out", [BLOC, COUT, OUTW], bf16, kind="ExternalOutput")

    with tile.TileContext(nc) as tc:
        with (
            tc.tile_pool(name="wpool", bufs=1) as wpool,
            tc.tile_pool(name="x0pool", bufs=1) as x0pool,
            tc.tile_pool(name="xpool", bufs=2) as xpool,
            tc.tile_pool(name="opool", bufs=4) as opool,
            tc.tile_pool(name="psum", bufs=1, space="PSUM") as psum,
            tc.tile_pool(name="wupsum", bufs=1, space="PSUM") as wupsum,
        ):
            wsb = wpool.tile([CIN, 9 * COUT], bf16)
            bsb = wpool.tile([128, 2], f32)
            # First trigger on the early-starting SYNC queue: the weights
            # the very first matmuls need (half 0, taps 0-2).
            nc.sync.dma_start(wsb[:, :384], w_d[:, :384])
            # Image-0 pieces follow on the same queue.
            x0 = []
            for p, (lo, hi) in enumerate(PBOUNDS):
                xt = x0pool.tile([CIN, hi - lo], bf16, tag=f"x0{p}", name=f"x0{p}")
                nc.sync.dma_start(xt[:], x_d[0, :, lo:hi])
                x0.append(xt)
            # Rest of the weights + bias on the ACT queue (starts ~1us
            # later, still well before tap 3 of the first group).
            nc.scalar.dma_start(wsb[:, 384:1152], w_d[:, 384:1152])
            nc.scalar.dma_start(wsb[:, 1152:], w_d[:, 1152:])
            nc.scalar.dma_start(bsb[:], b_d[:])

            # PE warmup (see module docstring).
            wub = wpool.tile([128, 512], bf16)
            nc.vector.memset(wub[:], 0.0)
            wup = wupsum.tile([128, 512], f32)
            for _ in range(WARMUP):
                nc.tensor.matmul(
                    wup[:], lhsT=wub[:, :128], rhs=wub[:],
                    start=True, stop=True,
                )

            def rhs(b, t, doff):
                # Moving operand for output tile t at tap offset doff.
                g = t * TILE_N + doff
                if b == 0:
                    p = t // 2
                    return x0[p][:, g - PBOUNDS[p][0] : g - PBOUNDS[p][0] + TILE_N]
                return xp[:, g : g + TILE_N]

            for b in range(BLOC):
                if b > 0:
                    xp = xpool.tile([CIN, NPIX], bf16, tag="xp", name="xp")
                    nc.sync.dma_start(xp[:], x_d[b, :, :])
                for h in range(2):
                    first = b == 0 and h == 0
                    last = b == BLOC - 1 and h == 1
                    # Compute sub-groups: pairs while image 0 streams in
                    # and for the final group (small kernel tail).
                    cgroups = (
                        [[0, 1], [2, 3], [4, 5], [6]]
                        if (first or last)
                        else [list(range(NTILES))]
                    )
                    # Output blocks (tiles, dma-queue engine).
                    if first:
                        oplan = [([0, 1], nc.gpsimd), ([2, 3], nc.sync),
                                 ([4, 5], nc.gpsimd), ([6], nc.sync)]
                    elif last:
                        oplan = [([0, 1], nc.sync), ([2, 3], nc.gpsimd),
                                 ([4, 5], nc.sync), ([6], nc.scalar)]
                    else:
                        qa, qb = (nc.gpsimd, nc.sync) if h == 0 else (nc.sync, nc.gpsimd)
                        oplan = [([0, 1, 2, 3], qa), ([4, 5, 6], qb)]

                    pts = {}
                    for grp in cgroups:
                        for t in grp:
                            pts[t] = psum.tile(
                                [128, TILE_N], f32, tag=f"pt{t}", name=f"pt{t}"
                            )
                        # Tap-outer: one stationary weight tile feeds all
                        # tiles of the group before the next tap loads.
                        for tap in range(9):
                            kh, kw = divmod(tap, 3)
                            c0 = h * (9 * 128) + tap * 128
                            for t in grp:
                                nc.tensor.matmul(
                                    pts[t][:],
                                    lhsT=wsb[:, c0 : c0 + 128],
                                    rhs=rhs(b, t, kh * SP + kw),
                                    start=(tap == 0),
                                    stop=(tap == 8),
                                )
                        # Drain this sub-group's finished blocks: bias-add
                        # PSUM -> SBUF bf16, even tiles on DVE, odd on ACT,
                        # then ship each completed output block.
                        for ts_, q in oplan:
                            if not all(t in grp for t in ts_):
                                continue
                            ot = opool.tile(
                                [128, 4 * TILE_N], bf16, tag="ot", name="ot"
                            )
                            for j, t in enumerate(ts_):
                                dst = ot[:, j * TILE_N : (j + 1) * TILE_N]
                                if t % 2 == 0:
                                    nc.vector.tensor_scalar_add(
                                        dst, pts[t][:], bsb[:, h : h + 1]
                                    )
                                else:
                                    nc.scalar.add(
                                        dst, pts[t][:], bsb[:, h : h + 1]
                                    )
                            w_out = len(ts_) * TILE_N
                            q.dma_start(
                                o_d[b, h * 128 : (h + 1) * 128,
                                    ts_[0] * TILE_N : ts_[0] * TILE_N + w_out],
                                ot[:, :w_out],
                            )
    nc.compile()
    return nc


def _get_nc():
    global _nc_cache
    if _nc_cache is None:
        _nc_cache = _build()
    return _nc_cache


def _prep_inputs(x, weights, bias):
    x = np.ascontiguousarray(np.asarray(x, dtype=np.float32))
    weights = np.ascontiguousarray(np.asarray(weights, dtype=np.float32))
    bias = np.ascontiguousarray(np.asarray(bias, dtype=np.float32))

    # Shared-padding stride-57 layout: rows -1..56 at stride 57 plus one
    # trailing zero (the last row's right pad) plus one slack zero.
    xpad = np.pad(x, ((0, 0), (0, 0), (1, 1), (1, 0))).reshape(B, CIN, (H + 2) * SP)
    xp = np.concatenate(
        [xpad, np.zeros((B, CIN, 2), dtype=np.float32)], axis=2
    ).astype(ml_dtypes.bfloat16)  # [B, CIN, 3308]
    # [Cout,Cin,3,3] -> [Cin, (half kh kw co)] so each Cout-half's taps are
    # one contiguous 1152-col block and each lhsT tap slice is contiguous.
    wT = np.ascontiguousarray(
        weights.reshape(2, 128, CIN, 3, 3).transpose(2, 0, 3, 4, 1)
    ).reshape(CIN, 9 * COUT).astype(ml_dtypes.bfloat16)
    b2 = np.ascontiguousarray(bias.reshape(2, 128).T)  # b2[p, h] = bias[h*128+p]

    return [
        {
            "xp": np.ascontiguousarray(xp[i * BLOC : (i + 1) * BLOC]),
            "wT": wT,
            "bias2": b2,
        }
        for i in range(NCORES)
    ]


def _run(inputs, trace=False):
    in_maps = _prep_inputs(inputs["x"], inputs["weights"], inputs["bias"])
    res = run_bass_kernel_spmd(
        _get_nc(), in_maps, core_ids=list(range(NCORES)), trace=trace
    )
    out = np.concatenate(
        [r["out"].astype(np.float32) for r in res.results], axis=0
    )  # [B, COUT, 3192]
    out = out.reshape(B, COUT, H, SP)[:, :, :, :W]
    return np.ascontiguousarray(out), res


def kernel(x, weights, bias):
    out, _ = _run({"x": x, "weights": weights, "bias": bias})
    return out


# revision 11
# speedup vs baseline: 1.0427x; 1.0027x over previous
"""Conv2d 3x3 (pad 1, stride 1) + bias on 8 Trainium2 cores.

Problem: x [32,128,56,56] f32, weights [256,128,3,3] f32, bias [256] f32
         -> out [32,256,56,56] f32.

Strategy
--------
Data-parallel over batch: each of the 8 cores owns 4 images.

Per core, implicit GEMM on a shared-padding row layout with stride 57:
  - Padded pixel (ih, iw), ih in [-1,56], iw in [-1,56], lives at flat
    index (ih+1)*57 + (iw+1); the right pad of row r IS the left pad of
    row r+1 (one shared zero column), so the buffer is 58*57+1(+1) = 3308
    bf16 per channel. The host builds this layout with np.pad.
  - Output is computed in the same stride-57 layout: out position
    p = oh*57 + ow. Every tap (kh, kw) of the 3x3 kernel is then a
    CONSTANT offset kh*57+kw into the flat padded input, so one matmul
    covers 8 output rows at once (N = 8*57 = 456 <= 512 PSUM bank).

bf16 + tap-outer (vs the f32r tap-inner baseline at 123.8us):
  - f32r matmuls are SELF-LOADING: the 128-cycle weight load serializes
    with every matmul (128/2.4GHz + 2.5ns NX = 55.8ns/mm, exactly the
    measured baseline gap).  bf16 matmuls emit a separate LDWEIGHTS that
    the PE's 64-deep reorder window pulls ahead into the background
    weight buffer while the previous matmul streams: measured steady
    state is 192-196 ns/matmul = the N/2.4GHz + NX roofline.
  - Tap-outer ordering: for each (image, cout-half) the 9 taps are the
    outer loop and the 7 output tiles the inner loop, so 7 consecutive
    matmuls share one stationary weight tile (7 PSUM banks live, bank t
    recycles exactly when the next group's tap-0 sweep reaches tile t).
  - bf16 end-to-end (hostside convert): max rel err ~3.5e-3 (vs 2e-2
    gate), and input/output DMA bytes halve.
  - PSUM tiles drain with the bias add fused: even tiles on DVE
    (tensor_scalar_add), odd tiles on ACT (activation Identity+bias), so
    the drain never gates the next group's first matmuls.

DMA schedule (16 shared SDMA engines, ~1us queue-start latency, ~300
GB/s aggregate; HWDGE rings on sync/scalar ~0.6us fixed, SWDGE on
gpsimd ~2us fixed):
  - Tile's dep tracking gates a tile's readers on ALL outstanding writes
    to it, so chunking one image DMA does NOT let early matmuls start
    early (measured: first matmul waited for the last chunk).  Image 0
    therefore lands as FOUR SEPARATE overlapping tiles (cols p*912 ..
    p*912+1144) and tile t reads piece t//2 only: the first matmuls
    gate on piece 0 alone.
  - The SYNC queue starts earliest after the ~7.2us framework preamble,
    so it carries w taps 0-2 of half 0 first, then image-0 pieces, then
    whole-image prefetches for images 1-3.  Remaining weights + bias go
    on the ACT queue.
  - (b=0, h=0) and the LAST (b,h) are computed in pair-groups; outputs
    ship per pair there, and as 2 blocks (tiles 0-3 / 4-6) otherwise,
    spread over the gpsimd and sync queues.  The last group's blocks go
    on HWDGE queues only (sync/sync/gpsimd-early/scalar-last) so the
    kernel tail is one small low-latency transfer.
  - PE warmup: a few bf16 throwaway matmuls on a memset tile keep the PE
    busy from t=0 so the HAM clock gate flips to 8/8 (2.4 GHz) ~3.4us
    after the first warmup while real (cold) matmuls make progress.

Built on bacc.Bacc: walrus engine-instruction structs hold at most ONE
sync wait; Bacc's compile() runs move_matmul_waits_to_ldweights /
generate_event_semaphores to split excess waits.
"""

import numpy as np
import ml_dtypes

import concourse.bacc as bacc
import concourse.mybir as mybir
import concourse.tile as tile
from concourse.bass_utils import run_bass_kernel_spmd

B, CIN, H, W = 32, 128, 56, 56
COUT = 256
NCORES = 8
BLOC = B // NCORES  # images per core
SP = W + 1  # 57: row stride of the shared-padding layout
# One trailing zero for the 58*57+1 layout plus one more so the last
# tile's bottom-right tap (offset 2*57+2) stays in bounds: 6*456+116+456.
NPIX = (H + 2) * SP + 2  # 3308 padded bf16 per channel
OUTW = H * SP  # 3192 output cols in stride-57 layout (1 junk col per row)
TILE_N = 8 * SP  # 456: 8 output rows per PSUM tile
NTILES = 7  # 7 x 456 = 3192
WARMUP = 4  # throwaway matmuls covering DMA-start latency on the PE
# Image-0 pieces: piece t holds cols [456t, 456t+572) -- exactly what
# output tile t reads (max tap offset 2*57+2 = 116; 456+116 = 572).
PBOUNDS = [(TILE_N * t, TILE_N * t + 572) for t in range(NTILES)]

_nc_cache = None


def _build():
    f32 = mybir.dt.float32
    bf16 = mybir.dt.bfloat16
    nc = bacc.Bacc("TRN2", target_bir_lowering=False)
    x_d = nc.dram_tensor("xp", [BLOC, CIN, NPIX], bf16, kind="ExternalInput")
    w_d = nc.dram_tensor("wT", [CIN, 9 * COUT], bf16, kind="ExternalInput")
    b_d = nc.dram_tensor("bias2", [128, 2], f32, kind="ExternalInput")
    o_d = nc.dram_tensor("out", [BLOC, COUT, OUTW], bf16, kind="ExternalOutput")

    with tile.TileContext(nc) as tc:
        with (
            tc.tile_pool(name="wpool", bufs=1) as wpool,
            tc.tile_pool(name="x0pool", bufs=1) as x0pool,
            tc.tile_pool(name="xpool", bufs=2) as xpool,
            tc.tile_pool(name="opool", bufs=4) as opool,
            tc.tile_pool(name="psum", bufs=1, space="PSUM") as psum,
            tc.tile_pool(name="wupsum", bufs=1, space="PSUM") as wupsum,
        ):
            wsb = wpool.tile([CIN, 9 * COUT], bf16)
            bsb = wpool.tile([128, 2], f32)
            # Everything the cold-start matmuls gate on rides the SYNC
            # HWDGE FIFO in exact need-order (the 16 SDMA engines are
            # shared between queues, so keeping competing traffic off
            # other queues during the ramp matters): taps 0-2 weights,
            # image-0 pieces 0-1, taps 3-8, remaining pieces, half-1
            # weights.  Only the (tiny, late-needed) bias uses ACT.
            nc.sync.dma_start(wsb[:, :384], w_d[:, :384])
            x0 = []
            for p, (lo, hi) in enumerate(PBOUNDS):
                xt = x0pool.tile([CIN, hi - lo], bf16, tag=f"x0{p}", name=f"x0{p}")
                x0.append(xt)
            nc.sync.dma_start(x0[0][:], x_d[0, :, PBOUNDS[0][0] : PBOUNDS[0][1]])
            nc.sync.dma_start(wsb[:, 384:1152], w_d[:, 384:1152])
            for p in range(1, NTILES):
                lo, hi = PBOUNDS[p]
                nc.sync.dma_start(x0[p][:], x_d[0, :, lo:hi])
            nc.sync.dma_start(wsb[:, 1152:], w_d[:, 1152:])
            nc.scalar.dma_start(bsb[:], b_d[:])

            # PE warmup (see module docstring).
            wub = wpool.tile([128, 512], bf16)
            nc.vector.memset(wub[:], 0.0)
            wup = wupsum.tile([128, 512], f32)
            for _ in range(WARMUP):
                nc.tensor.matmul(
                    wup[:], lhsT=wub[:, :128], rhs=wub[:],
                    start=True, stop=True,
                )

            def rhs(b, t, doff):
                # Moving operand for output tile t at tap offset doff.
                if b == 0:
                    return x0[t][:, doff : doff + TILE_N]
                return xp[:, t * TILE_N + doff : t * TILE_N + doff + TILE_N]

            for b in range(BLOC):
                if b > 0:
                    xp = xpool.tile([CIN, NPIX], bf16, tag="xp", name="xp")
                    nc.sync.dma_start(xp[:], x_d[b, :, :])
                for h in range(2):
                    first = b == 0 and h == 0
                    last = b == BLOC - 1 and h == 1
                    # Compute sub-groups: singletons while image 0
                    # streams in piece by piece, pairs for the final
                    # group (small kernel tail).
                    if first:
                        cgroups = [[t] for t in range(NTILES)]
                    elif last:
                        cgroups = [[0, 1], [2, 3], [4, 5], [6]]
                    else:
                        cgroups = [list(range(NTILES))]
                    # Output blocks (tiles, dma-queue engine).
                    if first:
                        oplan = [([t], nc.gpsimd if t % 2 == 0 else nc.sync)
                                 for t in range(NTILES)]
                    elif last:
                        oplan = [([0, 1], nc.sync), ([2, 3], nc.gpsimd),
                                 ([4, 5], nc.sync), ([6], nc.scalar)]
                    else:
                        qa, qb = (nc.gpsimd, nc.sync) if h == 0 else (nc.sync, nc.gpsimd)
                        oplan = [([0, 1, 2, 3], qa), ([4, 5, 6], qb)]

                    pts = {}
                    for grp in cgroups:
                        for t in grp:
                            pts[t] = psum.tile(
                                [128, TILE_N], f32, tag=f"pt{t}", name=f"pt{t}"
                            )
                        # Tap-outer: one stationary weight tile feeds all
                        # tiles of the group before the next tap loads.
                        for tap in range(9):
                            kh, kw = divmod(tap, 3)
                            c0 = h * (9 * 128) + tap * 128
                            for t in grp:
                                nc.tensor.matmul(
                                    pts[t][:],
                                    lhsT=wsb[:, c0 : c0 + 128],
                                    rhs=rhs(b, t, kh * SP + kw),
                                    start=(tap == 0),
                                    stop=(tap == 8),
                                )
                        # Drain this sub-group's finished blocks: bias-add
                        # PSUM -> SBUF bf16, even tiles on DVE, odd on ACT,
                        # then ship each completed output block.
                        for ts_, q in oplan:
                            if not all(t in grp for t in ts_):
                                continue
                            ot = opool.tile(
                                [128, 4 * TILE_N], bf16, tag="ot", name="ot"
                            )
                            for j, t in enumerate(ts_):
                                dst = ot[:, j * TILE_N : (j + 1) * TILE_N]
                                if t % 2 == 0:
                                    nc.vector.tensor_scalar_add(
                                        dst, pts[t][:], bsb[:, h : h + 1]
                                    )
                                else:
                                    nc.scalar.add(
                                        dst, pts[t][:], bsb[:, h : h + 1]
                                    )
                            w_out = len(ts_) * TILE_N
                            q.dma_start(
                                o_d[b, h * 128 : (h + 1) * 128,
                                    ts_[0] * TILE_N : ts_[0] * TILE_N + w_out],
                                ot[:, :w_out],
                            )
    nc.compile()
    return nc


def _get_nc():
    global _nc_cache
    if _nc_cache is None:
        _nc_cache = _build()
    return _nc_cache


def _prep_inputs(x, weights, bias):
    x = np.ascontiguousarray(np.asarray(x, dtype=np.float32))
    weights = np.ascontiguousarray(np.asarray(weights, dtype=np.float32))
    bias = np.ascontiguousarray(np.asarray(bias, dtype=np.float32))

    # Shared-padding stride-57 layout: rows -1..56 at stride 57 plus one
    # trailing zero (the last row's right pad) plus one slack zero.
    xpad = np.pad(x, ((0, 0), (0, 0), (1, 1), (1, 0))).reshape(B, CIN, (H + 2) * SP)
    xp = np.concatenate(
        [xpad, np.zeros((B, CIN, 2), dtype=np.float32)], axis=2
    ).astype(ml_dtypes.bfloat16)  # [B, CIN, 3308]
    # [Cout,Cin,3,3] -> [Cin, (half kh kw co)] so each Cout-half's taps are
    # one contiguous 1152-col block and each lhsT tap slice is contiguous.
    wT = np.ascontiguousarray(
        weights.reshape(2, 128, CIN, 3, 3).transpose(2, 0, 3, 4, 1)
    ).reshape(CIN, 9 * COUT).astype(ml_dtypes.bfloat16)
    b2 = np.ascontiguousarray(bias.reshape(2, 128).T)  # b2[p, h] = bias[h*128+p]

    return [
        {
            "xp": np.ascontiguousarray(xp[i * BLOC : (i + 1) * BLOC]),
            "wT": wT,
            "bias2": b2,
        }
        for i in range(NCORES)
    ]


def _run(inputs, trace=False):
    in_maps = _prep_inputs(inputs["x"], inputs["weights"], inputs["bias"])
    res = run_bass_kernel_spmd(
        _get_nc(), in_maps, core_ids=list(range(NCORES)), trace=trace
    )
    out = np.concatenate(
        [r["out"].astype(np.float32) for r in res.results], axis=0
    )  # [B, COUT, 3192]
    out = out.reshape(B, COUT, H, SP)[:, :, :, :W]
    return np.ascontiguousarray(out), res


def kernel(x, weights, bias):
    out, _ = _run({"x": x, "weights": weights, "bias": bias})
    return out


# revision 13
# speedup vs baseline: 1.0590x; 1.0156x over previous
"""Conv2d 3x3 (pad 1, stride 1) + bias on 8 Trainium2 cores.

Problem: x [32,128,56,56] f32, weights [256,128,3,3] f32, bias [256] f32
         -> out [32,256,56,56] f32.

Strategy
--------
Data-parallel over batch: each of the 8 cores owns 4 images.

Per core, implicit GEMM on a shared-padding row layout with stride 57:
  - Padded pixel (ih, iw), ih in [-1,56], iw in [-1,56], lives at flat
    index (ih+1)*57 + (iw+1); the right pad of row r IS the left pad of
    row r+1 (one shared zero column), so the buffer is 58*57+1(+1) = 3308
    bf16 per channel. The host builds this layout with np.pad.
  - Output is computed in the same stride-57 layout: out position
    p = oh*57 + ow. Every tap (kh, kw) of the 3x3 kernel is then a
    CONSTANT offset kh*57+kw into the flat padded input, so one matmul
    covers 8 output rows at once (N = 8*57 = 456 <= 512 PSUM bank).

bf16 + tap-outer (vs the f32r tap-inner baseline at 123.8us):
  - f32r matmuls are SELF-LOADING: the 128-cycle weight load serializes
    with every matmul (128/2.4GHz + 2.5ns NX = 55.8ns/mm, exactly the
    measured baseline gap).  bf16 matmuls emit a separate LDWEIGHTS that
    the PE's 64-deep reorder window pulls ahead into the background
    weight buffer while the previous matmul streams: measured steady
    state is 192-196 ns/matmul = the N/2.4GHz + NX roofline.
  - Tap-outer ordering: for each (image, cout-half) the 9 taps are the
    outer loop and the 7 output tiles the inner loop, so 7 consecutive
    matmuls share one stationary weight tile (7 PSUM banks live, bank t
    recycles exactly when the next group's tap-0 sweep reaches tile t).
  - bf16 end-to-end (hostside convert): max rel err ~3.5e-3 (vs 2e-2
    gate), and input/output DMA bytes halve.
  - PSUM tiles drain with the bias add fused: even tiles on DVE
    (tensor_scalar_add), odd tiles on ACT (activation Identity+bias), so
    the drain never gates the next group's first matmuls.

DMA schedule (16 shared SDMA engines, ~1us queue-start latency, ~300
GB/s aggregate; HWDGE rings on sync/scalar ~0.6us fixed, SWDGE on
gpsimd ~2us fixed):
  - Tile's dep tracking gates a tile's readers on ALL outstanding writes
    to it, so chunking one image DMA does NOT let early matmuls start
    early (measured: first matmul waited for the last chunk).  Image 0
    therefore lands as FOUR SEPARATE overlapping tiles (cols p*912 ..
    p*912+1144) and tile t reads piece t//2 only: the first matmuls
    gate on piece 0 alone.
  - The SYNC queue starts earliest after the ~7.2us framework preamble,
    so it carries w taps 0-2 of half 0 first, then image-0 pieces, then
    whole-image prefetches for images 1-3.  Remaining weights + bias go
    on the ACT queue.
  - (b=0, h=0) and the LAST (b,h) are computed in pair-groups; outputs
    ship per pair there, and as 2 blocks (tiles 0-3 / 4-6) otherwise,
    spread over the gpsimd and sync queues.  The last group's blocks go
    on HWDGE queues only (sync/sync/gpsimd-early/scalar-last) so the
    kernel tail is one small low-latency transfer.
  - PE warmup: a few bf16 throwaway matmuls on a memset tile keep the PE
    busy from t=0 so the HAM clock gate flips to 8/8 (2.4 GHz) ~3.4us
    after the first warmup while real (cold) matmuls make progress.

Built on bacc.Bacc: walrus engine-instruction structs hold at most ONE
sync wait; Bacc's compile() runs move_matmul_waits_to_ldweights /
generate_event_semaphores to split excess waits.
"""

import numpy as np
import ml_dtypes

import concourse.bacc as bacc
import concourse.mybir as mybir
import concourse.tile as tile
from concourse.bass_utils import run_bass_kernel_spmd

B, CIN, H, W = 32, 128, 56, 56
COUT = 256
NCORES = 8
BLOC = B // NCORES  # images per core
SP = W + 1  # 57: row stride of the shared-padding layout
# One trailing zero for the 58*57+1 layout plus one more so the last
# tile's bottom-right tap (offset 2*57+2) stays in bounds: 6*456+116+456.
NPIX = (H + 2) * SP + 2  # 3308 padded bf16 per channel
OUTW = H * SP  # 3192 output cols in stride-57 layout (1 junk col per row)
TILE_N = 8 * SP  # 456: 8 output rows per PSUM tile
NTILES = 7  # 7 x 456 = 3192
# Throwaway matmuls bridging the PE from t~8.4us (engine preamble done)
# to ~11us (first input piece + weights resident, incl. ~0.9us HWDGE
# completion-semaphore latency).  N=256 each: 213ns cold, so the bridge
# quantizes finely and the PE never idles long enough to reset the HAM
# activity window before the real stream begins.
WARMUP = 12
WARMUP_N = 256
# Image-0 pieces: piece t holds cols [456t, 456t+572) -- exactly what
# output tile t reads (max tap offset 2*57+2 = 116; 456+116 = 572).
PBOUNDS = [(TILE_N * t, TILE_N * t + 572) for t in range(NTILES)]

_nc_cache = None


def _build():
    f32 = mybir.dt.float32
    bf16 = mybir.dt.bfloat16
    nc = bacc.Bacc("TRN2", target_bir_lowering=False)
    x_d = nc.dram_tensor("xp", [BLOC, CIN, NPIX], bf16, kind="ExternalInput")
    w_d = nc.dram_tensor("wT", [CIN, 9 * COUT], bf16, kind="ExternalInput")
    b_d = nc.dram_tensor("bias2", [128, 2], f32, kind="ExternalInput")
    o_d = nc.dram_tensor("out", [BLOC, COUT, OUTW], bf16, kind="ExternalOutput")

    with tile.TileContext(nc) as tc:
        with (
            tc.tile_pool(name="wpool", bufs=1) as wpool,
            tc.tile_pool(name="x0pool", bufs=1) as x0pool,
            tc.tile_pool(name="xpool", bufs=2) as xpool,
            tc.tile_pool(name="opool", bufs=4) as opool,
            tc.tile_pool(name="psum", bufs=1, space="PSUM") as psum,
            tc.tile_pool(name="wupsum", bufs=1, space="PSUM") as wupsum,
        ):
            wsb = wpool.tile([CIN, 9 * COUT], bf16)
            bsb = wpool.tile([128, 2], f32)
            # Everything the cold-start matmuls gate on rides the SYNC
            # HWDGE FIFO in exact need-order (the 16 SDMA engines are
            # shared between queues, so keeping competing traffic off
            # other queues during the ramp matters): taps 0-2 weights,
            # image-0 pieces 0-1, taps 3-8, remaining pieces, half-1
            # weights.  Only the (tiny, late-needed) bias uses ACT.
            nc.sync.dma_start(wsb[:, :384], w_d[:, :384])
            x0 = []
            for p, (lo, hi) in enumerate(PBOUNDS):
                xt = x0pool.tile([CIN, hi - lo], bf16, tag=f"x0{p}", name=f"x0{p}")
                x0.append(xt)
            nc.sync.dma_start(x0[0][:], x_d[0, :, PBOUNDS[0][0] : PBOUNDS[0][1]])
            nc.sync.dma_start(wsb[:, 384:1152], w_d[:, 384:1152])
            for p in range(1, NTILES):
                lo, hi = PBOUNDS[p]
                nc.sync.dma_start(x0[p][:], x_d[0, :, lo:hi])
            nc.sync.dma_start(wsb[:, 1152:], w_d[:, 1152:])
            nc.scalar.dma_start(bsb[:], b_d[:])

            # PE warmup (see module docstring).
            wub = wpool.tile([128, WARMUP_N], bf16)
            nc.vector.memset(wub[:], 0.0)
            wup = wupsum.tile([128, WARMUP_N], f32)
            for _ in range(WARMUP):
                nc.tensor.matmul(
                    wup[:], lhsT=wub[:, :128], rhs=wub[:],
                    start=True, stop=True,
                )

            def rhs(b, t, doff):
                # Moving operand for output tile t at tap offset doff.
                if b == 0:
                    return x0[t][:, doff : doff + TILE_N]
                return xp[:, t * TILE_N + doff : t * TILE_N + doff + TILE_N]

            for b in range(BLOC):
                if b > 0:
                    xp = xpool.tile([CIN, NPIX], bf16, tag="xp", name="xp")
                    nc.sync.dma_start(xp[:], x_d[b, :, :])
                for h in range(2):
                    first = b == 0 and h == 0
                    last = b == BLOC - 1 and h == 1
                    # Compute sub-groups: singletons while image 0
                    # streams in piece by piece, pairs for the final
                    # group (small kernel tail).
                    if first:
                        cgroups = [[t] for t in range(NTILES)]
                    elif last:
                        cgroups = [[0, 1], [2, 3], [4, 5], [6]]
                    else:
                        cgroups = [list(range(NTILES))]
                    # Output blocks (tiles, dma-queue engine).
                    if first:
                        oplan = [([t], nc.gpsimd if t % 2 == 0 else nc.sync)
                                 for t in range(NTILES)]
                    elif last:
                        oplan = [([0, 1], nc.sync), ([2, 3], nc.gpsimd),
                                 ([4, 5], nc.sync), ([6], nc.scalar)]
                    else:
                        qa, qb = (nc.gpsimd, nc.sync) if h == 0 else (nc.sync, nc.gpsimd)
                        oplan = [([0, 1, 2, 3], qa), ([4, 5, 6], qb)]

                    pts = {}
                    for grp in cgroups:
                        for t in grp:
                            pts[t] = psum.tile(
                                [128, TILE_N], f32, tag=f"pt{t}", name=f"pt{t}"
                            )
                        # Tap-outer: one stationary weight tile feeds all
                        # tiles of the group before the next tap loads.
                        for tap in range(9):
                            kh, kw = divmod(tap, 3)
                            c0 = h * (9 * 128) + tap * 128
                            for t in grp:
                                nc.tensor.matmul(
                                    pts[t][:],
                                    lhsT=wsb[:, c0 : c0 + 128],
                                    rhs=rhs(b, t, kh * SP + kw),
                                    start=(tap == 0),
                                    stop=(tap == 8),
                                )
                        # Drain this sub-group's finished blocks: bias-add
                        # PSUM -> SBUF bf16, even tiles on DVE, odd on ACT,
                        # then ship each completed output block.
                        for ts_, q in oplan:
                            if not all(t in grp for t in ts_):
                                continue
                            ot = opool.tile(
                                [128, 4 * TILE_N], bf16, tag="ot", name="ot"
                            )
                            for j, t in enumerate(ts_):
                                dst = ot[:, j * TILE_N : (j + 1) * TILE_N]
                                if t % 2 == 0:
                                    nc.vector.tensor_scalar_add(
                                        dst, pts[t][:], bsb[:, h : h + 1]
                                    )
                                else:
                                    nc.scalar.add(
                                        dst, pts[t][:], bsb[:, h : h + 1]
                                    )
                            w_out = len(ts_) * TILE_N
                            q.dma_start(
                                o_d[b, h * 128 : (h + 1) * 128,
                                    ts_[0] * TILE_N : ts_[0] * TILE_N + w_out],
                                ot[:, :w_out],
                            )
    nc.compile()
    return nc


def _get_nc():
    global _nc_cache
    if _nc_cache is None:
        _nc_cache = _build()
    return _nc_cache


def _prep_inputs(x, weights, bias):
    x = np.ascontiguousarray(np.asarray(x, dtype=np.float32))
    weights = np.ascontiguousarray(np.asarray(weights, dtype=np.float32))
    bias = np.ascontiguousarray(np.asarray(bias, dtype=np.float32))

    # Shared-padding stride-57 layout: rows -1..56 at stride 57 plus one
    # trailing zero (the last row's right pad) plus one slack zero.
    xpad = np.pad(x, ((0, 0), (0, 0), (1, 1), (1, 0))).reshape(B, CIN, (H + 2) * SP)
    xp = np.concatenate(
        [xpad, np.zeros((B, CIN, 2), dtype=np.float32)], axis=2
    ).astype(ml_dtypes.bfloat16)  # [B, CIN, 3308]
    # [Cout,Cin,3,3] -> [Cin, (half kh kw co)] so each Cout-half's taps are
    # one contiguous 1152-col block and each lhsT tap slice is contiguous.
    wT = np.ascontiguousarray(
        weights.reshape(2, 128, CIN, 3, 3).transpose(2, 0, 3, 4, 1)
    ).reshape(CIN, 9 * COUT).astype(ml_dtypes.bfloat16)
    b2 = np.ascontiguousarray(bias.reshape(2, 128).T)  # b2[p, h] = bias[h*128+p]

    return [
        {
            "xp": np.ascontiguousarray(xp[i * BLOC : (i + 1) * BLOC]),
            "wT": wT,
            "bias2": b2,
        }
        for i in range(NCORES)
    ]


def _run(inputs, trace=False):
    in_maps = _prep_inputs(inputs["x"], inputs["weights"], inputs["bias"])
    res = run_bass_kernel_spmd(
        _get_nc(), in_maps, core_ids=list(range(NCORES)), trace=trace
    )
    out = np.concatenate(
        [r["out"].astype(np.float32) for r in res.results], axis=0
    )  # [B, COUT, 3192]
    out = out.reshape(B, COUT, H, SP)[:, :, :, :W]
    return np.ascontiguousarray(out), res


def kernel(x, weights, bias):
    out, _ = _run({"x": x, "weights": weights, "bias": bias})
    return out
